# Initial kernel scaffold
#
"""Trainium2 Bass kernel for masked tanh-clipped attention softmax.

Reference computation (B=16, NQ=NK=2048, KD=QD=KQ=256, CLIP=10):
    k = k_inputs @ Wk                     [B, NK, 256]
    q = q_inputs @ Wq                     [B, NQ, 256]
    s = (q @ k^T) / 16                    [B, NQ, NK]
    s = tanh(s) * 10
    s = where(adjancy, s, -inf)
    out = softmax(s, axis=2)

Kernel strategy (per NeuronCore, 2 batches each across 8 cores):
  * Fold the projections: A = Wq @ Wk^T (256x256), so s = q_in @ A @ k_in^T
    (fp16 matmul; A rounded to fp16).
  * Host passes q_in/k_in pre-transposed to [d, token] fp16, adjacency as
    uint8 scaled to {0,2}, Wq^T/Wk^T fp16.
  * qaT = A^T @ q_inT; first chunk upfront, the rest threaded one chunk per
    m-tile through the PE's slack so the ACT pipeline never stalls.
  * Per m-tile, software-pipelined by one tile so ACT runs back-to-back:
      ACT: t = tanh(s/16)   (PSUM -> SBUF fp16)
      DVE: t += mask16      (mask16 = Pool-converted {0,2} fp16)
      ACT: e = exp(10t - 20)       [masked entries <= e^-17: negligible]
      DVE: rowsum via tensor_scalar accum_out (4x mode), reciprocal,
           e *= 1/rowsum
  * adjacency in / out DMAs move 2 m-tiles per descriptor set (SP dispatch
    cost is flat 650ns per DMACopy instruction).
  * Output fp16 to HBM; host upcasts to f32.
  Engine busy per core: ACT ~121us (bottleneck: 2 unavoidable table passes
  over 8.4M elems at 0.833 ns/elem), Pool ~55, DVE ~85, PE ~65, DMA ~83.
"""
import numpy as np

import concourse.bacc as bacc
import concourse.mybir as mybir
from concourse.tile import TileContext
from concourse.bass_utils import run_bass_kernel_spmd

F32 = mybir.dt.float32
F16 = mybir.dt.float16
U8 = mybir.dt.uint8
AF = mybir.ActivationFunctionType
ALU = mybir.AluOpType

B, NQ, NK = 16, 2048, 2048
D = 256                 # KD = QD = KQ
CORES = 8
BPC = B // CORES        # batches per core
MT = 128                # query rows per tile
NMT = NQ // MT          # 16 m-tiles per batch
CH = 512                # psum bank free-dim (fp32)
NCH = NK // CH          # 4 n-chunks per scores row


def build(reps=1):
    nc = bacc.Bacc(None, target_bir_lowering=False)

    qT = nc.dram_tensor("qT", [BPC, D, NQ], F16, kind="ExternalInput")
    kT = nc.dram_tensor("kT", [BPC, D, NK], F16, kind="ExternalInput")
    adj = nc.dram_tensor("adj", [BPC, NQ, NK], U8, kind="ExternalInput")
    # A = Wq @ Wk^T, folded on host (weight preprocessing), laid out as
    # A_in[p, dc, e] = A[dc*128 + p, e]
    a_in = nc.dram_tensor("a_in", [D, D], F16, kind="ExternalInput")
    out = nc.dram_tensor("out", [BPC, NQ, NK], F16, kind="ExternalOutput")

    with TileContext(nc) as tc:
        with (
            tc.tile_pool(name="const", bufs=1) as cp,
            tc.tile_pool(name="mt", bufs=4) as mp,
            tc.tile_pool(name="pair", bufs=3) as pp,
            tc.tile_pool(name="ps", bufs=2, space="PSUM") as ps,
        ):
            batches = sorted(set(bb for _ in range(reps) for bb in range(BPC)))

            # ---- batch-0 operands first: they gate the first tile ----
            a_t = cp.tile([128, 2, D], F16)    # a_t[p, dc, e] = A[dc*128+p, e]
            nc.sync.dma_start(out=a_t[:], in_=a_in.rearrange("(c p) e -> p c e", p=128))
            qT_ts, kT_ts, qa_ts = {}, {}, {}
            for b in batches:
                qT_ts[b] = cp.tile([128, 2, NQ], F16, name=f"qT{b}")
                kT_ts[b] = cp.tile([128, 2, NK], F16, name=f"kT{b}")
                qa_ts[b] = cp.tile([128, 2, NQ], F16, name=f"qa{b}")
            # DMAs serialize at data rate; order by when tile 0 needs them:
            # qT0 chunk 0 (feeds first qa), full kT0, then the rest streams
            # behind the tile loop.
            b0 = batches[0]
            for dc in range(2):
                nc.sync.dma_start(out=qT_ts[b0][:, dc, 0:CH],
                                  in_=qT[b0, dc * 128:(dc + 1) * 128, 0:CH])
            for dc in range(2):
                nc.sync.dma_start(out=kT_ts[b0][:, dc],
                                  in_=kT[b0, dc * 128:(dc + 1) * 128, :])

            # warm up the PE p-state with dummy matmuls on a memset tile so
            # tile 0's matmuls run at full clock
            warm = cp.tile([128, CH], F16)
            nc.gpsimd.memset(warm[:], 0)
            warm_ps = ps.tile([128, CH], F32, tag="sc", name="warm_ps")
            for _ in range(14):
                nc.tensor.matmul(warm_ps[:], warm[:, 0:128], warm[:],
                                 start=True, stop=True)

            def late_loads(i):
                b0 = batches[0]
                if i == 0:
                    for dc in range(2):
                        nc.sync.dma_start(out=qT_ts[b0][:, dc, CH:],
                                          in_=qT[b0, dc * 128:(dc + 1) * 128, CH:])
                elif i == 1 and len(batches) > 1:
                    b1 = batches[1]
                    nc.sync.dma_start(
                        out=qT_ts[b1][:], in_=qT[b1].rearrange("(c p) m -> p c m", p=128))
                elif i == 2 and len(batches) > 1:
                    b1 = batches[1]
                    nc.sync.dma_start(
                        out=kT_ts[b1][:], in_=kT[b1].rearrange("(c p) m -> p c m", p=128))

            ebias = cp.tile([128, 1], F32)
            nc.vector.memset(ebias[:], -20.0)

            def qa_chunk(b, dp, mc):
                qa_ps = ps.tile([128, CH], F32, tag="sc", name="qa_ps")
                for dc in range(2):
                    nc.tensor.matmul(
                        qa_ps[:],
                        a_t[:, dc, dp * 128:(dp + 1) * 128],
                        qT_ts[b][:, dc, mc * CH:(mc + 1) * CH],
                        start=(dc == 0),
                        stop=(dc == 1),
                    )
                nc.vector.tensor_copy(
                    qa_ts[b][:, dp, mc * CH:(mc + 1) * CH], qa_ps[:])

            qa_jobs = [(b, dp, mc) for b in batches for mc in range(NCH)
                       for dp in range(2)]
            qa_chunk(*qa_jobs[0])
            qa_chunk(*qa_jobs[1])
            qa_left = qa_jobs[2:]

            # ---- flat software-pipelined m-tile loop over all batches ----
            # Depth-1 pipeline: ACT runs tanh(i), exp(i-1), tanh(i+1), ... so
            # it never waits on the DVE mask-add of the tile it just tanh'd.
            tiles = [(b, mt) for _ in range(reps) for b in batches
                     for mt in range(NMT)]
            pair = None
            prev = None
            for idx, (b, mt) in enumerate(tiles):
                half = idx % 2
                if half == 0:
                    # prefetch + convert masks for two tiles at once
                    adj_t = pp.tile([128, 2, NK], U8)
                    nc.sync.dma_start(
                        out=adj_t[:],
                        in_=adj[b, mt * MT:(mt + 2) * MT, :].rearrange(
                            "(t p) n -> p t n", p=128))
                    m16 = pp.tile([128, 2, NK], F16)
                    nc.gpsimd.tensor_copy(m16[:], adj_t[:])
                    e_pr = pp.tile([128, 2, NK], F16, name="e_pr")
                    pair = {"m16": m16, "e": e_pr, "b": b, "mt0": mt}
                my_pair = pair
                # scores matmuls
                sc_ps = ps.tile([128, NK], F32, tag="sc", name="sc_ps")
                for dp in range(2):
                    for n in range(NCH):
                        nc.tensor.matmul(
                            sc_ps[:, n * CH:(n + 1) * CH],
                            qa_ts[b][:, dp, mt * MT:(mt + 1) * MT],
                            kT_ts[b][:, dp, n * CH:(n + 1) * CH],
                            start=(dp == 0),
                            stop=(dp == 1),
                        )
                if qa_left and idx >= 1:
                    qa_chunk(*qa_left.pop(0))
                t_t = mp.tile([128, NK], F16)
                nc.scalar.activation(t_t[:], sc_ps[:], AF.Tanh, scale=1.0 / 16.0)
                # previous tile's epilogue keeps ACT busy during this DVE add
                if prev is not None:
                    pt, pp_, ph, prs, prc, pb, pmt = prev
                    nc.scalar.activation(pp_["e"][:, ph], pt[:], AF.Exp,
                                         scale=10.0, bias=ebias[:])
                nc.vector.tensor_tensor(t_t[:], t_t[:], my_pair["m16"][:, half],
                                        op=ALU.add)
                if prev is not None:
                    nc.vector.tensor_scalar(pt[:], pp_["e"][:, ph], 1.0, 0.0,
                                            op0=ALU.mult, op1=ALU.add,
                                            accum_out=prs[:])
                    nc.vector.reciprocal(prc[:], prs[:])
                    nc.vector.tensor_scalar_mul(pp_["e"][:, ph], pp_["e"][:, ph],
                                                prc[:])
                    if ph == 1:
                        nc.sync.dma_start(
                            out=out[pb, (pmt - 1) * MT:(pmt + 1) * MT, :].rearrange(
                                "(t p) n -> p t n", p=128),
                            in_=pp_["e"][:])
                    elif idx == len(tiles) - 1:
                        # drain: don't hold half 0 hostage to half 1
                        nc.sync.dma_start(
                            out=out[pb, pmt * MT:(pmt + 1) * MT, :],
                            in_=pp_["e"][:, 0])
                rsum = mp.tile([128, 1], F32, bufs=2)
                rcp = mp.tile([128, 1], F32, bufs=2)
                prev = (t_t, my_pair, half, rsum, rcp, b, mt)
                late_loads(idx)
            pt, pp_, ph, prs, prc, pb, pmt = prev
            nc.scalar.activation(pp_["e"][:, ph], pt[:], AF.Exp, scale=10.0,
                                 bias=ebias[:])
            nc.vector.tensor_scalar(pt[:], pp_["e"][:, ph], 1.0, 0.0,
                                    op0=ALU.mult, op1=ALU.add, accum_out=prs[:])
            nc.vector.reciprocal(prc[:], prs[:])
            nc.vector.tensor_scalar_mul(pp_["e"][:, ph], pp_["e"][:, ph], prc[:])
            nc.sync.dma_start(
                out=out[pb, pmt * MT:(pmt + 1) * MT, :], in_=pp_["e"][:, 1])
    nc.compile()
    return nc


_NC = None


def _get_nc():
    global _NC
    if _NC is None:
        _NC = build()
    return _NC


def kernel(k_inputs, q_inputs, adjancy, Wk, Wq):
    k_inputs = np.asarray(k_inputs, dtype=np.float32)
    q_inputs = np.asarray(q_inputs, dtype=np.float32)
    adjancy = np.asarray(adjancy, dtype=np.int32)
    Wk = np.asarray(Wk, dtype=np.float32)
    Wq = np.asarray(Wq, dtype=np.float32)
    nc = _get_nc()
    a_in = (Wq @ Wk.T).astype(np.float16)
    in_maps = []
    for c in range(CORES):
        lo, hi = c * BPC, (c + 1) * BPC
        in_maps.append({
            "qT": np.ascontiguousarray(
                q_inputs[lo:hi].transpose(0, 2, 1)).astype(np.float16),
            "kT": np.ascontiguousarray(
                k_inputs[lo:hi].transpose(0, 2, 1)).astype(np.float16),
            "adj": (adjancy[lo:hi] * 2).astype(np.uint8),
            "a_in": a_in,
        })
    res = run_bass_kernel_spmd(nc, in_maps, core_ids=list(range(CORES)))
    return np.concatenate(
        [res.results[c]["out"] for c in range(CORES)], axis=0
    ).astype(np.float32)



# revision 1
# speedup vs baseline: 2.5300x; 2.5300x over previous
"""Trainium2 Bass kernel for masked tanh-clipped attention softmax.

Reference computation (B=16, NQ=NK=2048, KD=QD=KQ=256, CLIP=10):
    k = k_inputs @ Wk                     [B, NK, 256]
    q = q_inputs @ Wq                     [B, NQ, 256]
    s = (q @ k^T) / 16                    [B, NQ, NK]
    s = tanh(s) * 10
    s = where(adjancy, s, -inf)
    out = softmax(s, axis=2)

Kernel strategy (per NeuronCore, 2 batches each across 8 cores):
  * Fold the projections: A = Wq @ Wk^T (256x256), so s = q_in @ A @ k_in^T
    (fp16 matmul; A rounded to fp16).
  * Host passes q_in/k_in pre-transposed to [d, token] fp16, adjacency as
    uint8 scaled to {0,2}, Wq^T/Wk^T fp16.
  * qaT = A^T @ q_inT; first chunk upfront, the rest threaded one chunk per
    m-tile through the PE's slack so the ACT pipeline never stalls.
  * Per m-tile, software-pipelined by one tile so ACT runs back-to-back:
      ACT: t = tanh(s/16)   (PSUM -> SBUF fp16)
      DVE: t += mask16      (mask16 = Pool-converted {0,2} fp16)
      ACT: e = exp(10t - 20)       [masked entries <= e^-17: negligible]
      DVE: rowsum via tensor_scalar accum_out (4x mode), reciprocal,
           e *= 1/rowsum
  * adjacency in / out DMAs move 2 m-tiles per descriptor set (SP dispatch
    cost is flat 650ns per DMACopy instruction).
  * Output fp16 to HBM; host upcasts to f32.
  Engine busy per core: ACT ~121us (bottleneck: 2 unavoidable table passes
  over 8.4M elems at 0.833 ns/elem), Pool ~55, DVE ~85, PE ~65, DMA ~83.
"""
import numpy as np

import concourse.bacc as bacc
import concourse.mybir as mybir
from concourse.tile import TileContext
from concourse.bass_utils import run_bass_kernel_spmd

F32 = mybir.dt.float32
F16 = mybir.dt.float16
U8 = mybir.dt.uint8
AF = mybir.ActivationFunctionType
ALU = mybir.AluOpType

B, NQ, NK = 16, 2048, 2048
D = 256                 # KD = QD = KQ
CORES = 8
BPC = B // CORES        # batches per core
MT = 128                # query rows per tile
NMT = NQ // MT          # 16 m-tiles per batch
CH = 512                # psum bank free-dim (fp32)
NCH = NK // CH          # 4 n-chunks per scores row


def build(reps=1):
    nc = bacc.Bacc(None, target_bir_lowering=False)

    qT = nc.dram_tensor("qT", [BPC, D, NQ], F16, kind="ExternalInput")
    kT = nc.dram_tensor("kT", [BPC, D, NK], F16, kind="ExternalInput")
    adj = nc.dram_tensor("adj", [BPC, NQ, NK], U8, kind="ExternalInput")
    # A = Wq @ Wk^T, folded on host (weight preprocessing), laid out as
    # A_in[p, dc, e] = A[dc*128 + p, e]
    a_in = nc.dram_tensor("a_in", [D, D], F16, kind="ExternalInput")
    out = nc.dram_tensor("out", [BPC, NQ, NK], F16, kind="ExternalOutput")

    with TileContext(nc) as tc:
        with (
            tc.tile_pool(name="const", bufs=1) as cp,
            tc.tile_pool(name="mt", bufs=4) as mp,
            tc.tile_pool(name="pair", bufs=3) as pp,
            tc.tile_pool(name="ps", bufs=2, space="PSUM") as ps,
        ):
            batches = sorted(set(bb for _ in range(reps) for bb in range(BPC)))

            # ---- batch-0 operands first: they gate the first tile ----
            a_t = cp.tile([128, 2, D], F16)    # a_t[p, dc, e] = A[dc*128+p, e]
            nc.sync.dma_start(out=a_t[:], in_=a_in.rearrange("(c p) e -> p c e", p=128))
            qT_ts, kT_ts, qa_ts = {}, {}, {}
            for b in batches:
                qT_ts[b] = cp.tile([128, 2, NQ], F16, name=f"qT{b}")
                kT_ts[b] = cp.tile([128, 2, NK], F16, name=f"kT{b}")
                qa_ts[b] = cp.tile([128, 2, NQ], F16, name=f"qa{b}")
            # DMAs serialize at data rate; order by when tile 0 needs them:
            # qT0 chunk 0 (feeds first qa), full kT0, then the rest streams
            # behind the tile loop.
            b0 = batches[0]
            for dc in range(2):
                nc.sync.dma_start(out=qT_ts[b0][:, dc, 0:CH],
                                  in_=qT[b0, dc * 128:(dc + 1) * 128, 0:CH])
            for dc in range(2):
                nc.sync.dma_start(out=kT_ts[b0][:, dc],
                                  in_=kT[b0, dc * 128:(dc + 1) * 128, :])

            # warm up the PE p-state with dummy matmuls on a memset tile so
            # tile 0's matmuls run at full clock
            warm = cp.tile([128, CH], F16)
            nc.gpsimd.memset(warm[:], 0)
            warm_ps = ps.tile([128, CH], F32, tag="sc", name="warm_ps")
            for _ in range(14):
                nc.tensor.matmul(warm_ps[:], warm[:, 0:128], warm[:],
                                 start=True, stop=True)

            def late_loads(i):
                b0 = batches[0]
                if i == 0:
                    for dc in range(2):
                        nc.sync.dma_start(out=qT_ts[b0][:, dc, CH:],
                                          in_=qT[b0, dc * 128:(dc + 1) * 128, CH:])
                elif i == 1 and len(batches) > 1:
                    b1 = batches[1]
                    nc.sync.dma_start(
                        out=qT_ts[b1][:], in_=qT[b1].rearrange("(c p) m -> p c m", p=128))
                elif i == 2 and len(batches) > 1:
                    b1 = batches[1]
                    nc.sync.dma_start(
                        out=kT_ts[b1][:], in_=kT[b1].rearrange("(c p) m -> p c m", p=128))

            ebias = cp.tile([128, 1], F32)
            nc.vector.memset(ebias[:], -20.0)

            def qa_chunk(b, dp, mc):
                qa_ps = ps.tile([128, CH], F32, tag="sc", name="qa_ps")
                for dc in range(2):
                    nc.tensor.matmul(
                        qa_ps[:],
                        a_t[:, dc, dp * 128:(dp + 1) * 128],
                        qT_ts[b][:, dc, mc * CH:(mc + 1) * CH],
                        start=(dc == 0),
                        stop=(dc == 1),
                    )
                nc.vector.tensor_copy(
                    qa_ts[b][:, dp, mc * CH:(mc + 1) * CH], qa_ps[:])

            qa_jobs = [(b, dp, mc) for b in batches for mc in range(NCH)
                       for dp in range(2)]
            qa_chunk(*qa_jobs[0])
            qa_chunk(*qa_jobs[1])
            qa_left = qa_jobs[2:]

            # ---- flat software-pipelined m-tile loop over all batches ----
            # Depth-1 pipeline: ACT runs tanh(i), exp(i-1), tanh(i+1), ... so
            # it never waits on the DVE mask-add of the tile it just tanh'd.
            tiles = [(b, mt) for _ in range(reps) for b in batches
                     for mt in range(NMT)]
            pair = None
            prev = None
            for idx, (b, mt) in enumerate(tiles):
                half = idx % 2
                if half == 0:
                    # prefetch + convert masks for two tiles at once
                    adj_t = pp.tile([128, 2, NK], U8)
                    nc.sync.dma_start(
                        out=adj_t[:],
                        in_=adj[b, mt * MT:(mt + 2) * MT, :].rearrange(
                            "(t p) n -> p t n", p=128))
                    m16 = pp.tile([128, 2, NK], F16)
                    nc.gpsimd.tensor_copy(m16[:], adj_t[:])
                    e_pr = pp.tile([128, 2, NK], F16, name="e_pr")
                    pair = {"m16": m16, "e": e_pr, "b": b, "mt0": mt}
                my_pair = pair
                # scores matmuls
                sc_ps = ps.tile([128, NK], F32, tag="sc", name="sc_ps")
                for dp in range(2):
                    for n in range(NCH):
                        nc.tensor.matmul(
                            sc_ps[:, n * CH:(n + 1) * CH],
                            qa_ts[b][:, dp, mt * MT:(mt + 1) * MT],
                            kT_ts[b][:, dp, n * CH:(n + 1) * CH],
                            start=(dp == 0),
                            stop=(dp == 1),
                        )
                if qa_left and idx >= 1:
                    qa_chunk(*qa_left.pop(0))
                t_t = mp.tile([128, NK], F16)
                nc.scalar.activation(t_t[:], sc_ps[:], AF.Tanh, scale=1.0 / 16.0)
                # previous tile's epilogue keeps ACT busy during this DVE add
                if prev is not None:
                    pt, pp_, ph, prs, prc, pb, pmt = prev
                    nc.scalar.activation(pp_["e"][:, ph], pt[:], AF.Exp,
                                         scale=10.0, bias=ebias[:])
                nc.vector.tensor_tensor(t_t[:], t_t[:], my_pair["m16"][:, half],
                                        op=ALU.add)
                if prev is not None:
                    nc.vector.tensor_scalar(pt[:], pp_["e"][:, ph], 1.0, 0.0,
                                            op0=ALU.mult, op1=ALU.add,
                                            accum_out=prs[:])
                    nc.vector.reciprocal(prc[:], prs[:])
                    nc.vector.tensor_scalar_mul(pp_["e"][:, ph], pp_["e"][:, ph],
                                                prc[:])
                    if ph == 1:
                        nc.sync.dma_start(
                            out=out[pb, (pmt - 1) * MT:(pmt + 1) * MT, :].rearrange(
                                "(t p) n -> p t n", p=128),
                            in_=pp_["e"][:])
                    elif idx == len(tiles) - 1:
                        # drain: don't hold half 0 hostage to half 1
                        nc.sync.dma_start(
                            out=out[pb, pmt * MT:(pmt + 1) * MT, :],
                            in_=pp_["e"][:, 0])
                rsum = mp.tile([128, 1], F32, bufs=2)
                rcp = mp.tile([128, 1], F32, bufs=2)
                prev = (t_t, my_pair, half, rsum, rcp, b, mt)
                late_loads(idx)
            pt, pp_, ph, prs, prc, pb, pmt = prev
            nc.scalar.activation(pp_["e"][:, ph], pt[:], AF.Exp, scale=10.0,
                                 bias=ebias[:])
            nc.vector.tensor_scalar(pt[:], pp_["e"][:, ph], 1.0, 0.0,
                                    op0=ALU.mult, op1=ALU.add, accum_out=prs[:])
            nc.vector.reciprocal(prc[:], prs[:])
            nc.vector.tensor_scalar_mul(pp_["e"][:, ph], pp_["e"][:, ph], prc[:])
            nc.sync.dma_start(
                out=out[pb, pmt * MT:(pmt + 1) * MT, :], in_=pp_["e"][:, 1])
    nc.compile()
    return nc


_NC = None


def _get_nc():
    global _NC
    if _NC is None:
        _NC = build()
    return _NC


def kernel(k_inputs, q_inputs, adjancy, Wk, Wq):
    k_inputs = np.asarray(k_inputs, dtype=np.float32)
    q_inputs = np.asarray(q_inputs, dtype=np.float32)
    adjancy = np.asarray(adjancy, dtype=np.int32)
    Wk = np.asarray(Wk, dtype=np.float32)
    Wq = np.asarray(Wq, dtype=np.float32)
    nc = _get_nc()
    a_in = (Wq @ Wk.T).astype(np.float16)
    in_maps = []
    for c in range(CORES):
        lo, hi = c * BPC, (c + 1) * BPC
        in_maps.append({
            "qT": np.ascontiguousarray(
                q_inputs[lo:hi].transpose(0, 2, 1)).astype(np.float16),
            "kT": np.ascontiguousarray(
                k_inputs[lo:hi].transpose(0, 2, 1)).astype(np.float16),
            "adj": (adjancy[lo:hi] * 2).astype(np.uint8),
            "a_in": a_in,
        })
    res = run_bass_kernel_spmd(nc, in_maps, core_ids=list(range(CORES)))
    return np.concatenate(
        [res.results[c]["out"] for c in range(CORES)], axis=0
    ).astype(np.float32)

